# revision 53
# baseline (speedup 1.0000x reference)
"""GP posterior mean mu = K_rbf(X_test, X_train) @ alpha on 8 NeuronCores,
exploiting the locality of the RBF kernel (lengthscale 0.1 on N(0,1) data).

Math per block: K[j,i] = sf2 * exp(-0.5*||xt_i - x_j||^2 / ell2), with the
exponent expressed as a single 14-term dot product built from bf16 hi/lo
splits of the fp32 operands, zero-padded to a 128-deep contraction.  The
padding is NOT for streaming speed (a small-K matmul streams at the same
N-cycles rate) -- it keeps the PE HAM activity monitor fed: a 16-row
contraction reads as a near-idle array and the clock gate locks the PE at
1.2 GHz instead of 2.4 GHz (measured 2x on every matmul).  ScalarE applies
exp (sf2 folded into the activation bias), and a second TensorE matmul
contracts K against hi/lo-split alpha, accumulating in PSUM.

Sparsity: test points are sorted into 64 compact spatial chunks of 256 by
recursive median bisection (host side).  For each chunk, only the train
points whose distance to the chunk's bbox keeps the RBF exponent above -TAU
are gathered (sorted near-to-far) and packed densely into per-chunk 128-point
contraction tiles.  The SPMD program is a uniform grid of S slots x T
train-tiles per core; heavy chunks are split across slots (partial sums
merged on host) and the remainder is padded with a zero-feature sentinel
column (exp -> 1, alpha 0), so every core runs the identical instruction
stream and only the gathered tile DATA differs per core.

Schedule learned from HW traces:
- Few, ordered DMAs: per-dma_start queue cost is ~0.6us and the 16 HW rings
  drain transfers in enqueue order, so slot 0's group goes first (tiny) and
  alpha rides a different trigger queue to bypass the bulk input FIFO.
- ~3.6us of junk matmuls on a zeroed tile run during the input DMA to flip
  the HAM clock gate before real work starts.
- One output buffer, two DMAs (the bulk leaves while the last slot runs).
- A fixed ~10us runtime epilogue (semaphore drain storm + final barrier)
  exists regardless of program content; a do-nothing program measures
  ~13.8us through this same path.
"""

import math

import numpy as np
import ml_dtypes

M = 16384
N = 16384
NCORES = 8
TC = 256                  # test points per chunk (columns per tile)
TT = 128                  # train points per tile (one PE contraction)
TAU = 3.2                 # drop blocks with min exponent magnitude > TAU
G = 6                     # train tiles covered by one ACT instruction
C = 14                    # used contraction rows of the exponent matmul
KP = 128                  # shipped contraction rows (128: a small-K matmul
                          # starves the PE HAM activity monitor and locks the
                          # clock at 1.2 GHz, so pad the contraction to 128)
NDMA = 5                  # input DMA groups (first kept tiny for fast start)
NWARM = 18                # junk matmuls issued during the input DMA to flip
                          # the PE clock gate to 2.4 GHz before real work.
                          # Must cover the full ~3.4us HAM window by itself:
                          # shortening to 12 (hoping slot 0's matmuls finish
                          # the streak) left the whole stream at 1.2 GHz
                          # (measured +7us) -- the idle gap before slot 0's
                          # data resets the activity window
THETA = 3.5               # tiles whose points all sit beyond this exponent
                          # magnitude get a DVE fast-exp instead of ScalarE.
                          # Set to TAU = disabled: measured on HW, the DVE
                          # offload never beat ScalarE because TensorE is
                          # equally saturated (~22us) and the coarse DVE
                          # instruction granularity added coupling stalls
C_SCH = 0.0434            # Schraudolph minimax offset for the fast exp
A_SCH = 128.0 * 1.4426950408889634      # 2^7 * log2(e)

_cache = {}


def _split2(v):
    hi = v.astype(ml_dtypes.bfloat16)
    lo = (v - hi.astype(np.float64)).astype(ml_dtypes.bfloat16)
    return hi, lo


def _split3(v):
    hi = v.astype(ml_dtypes.bfloat16)
    r = v - hi.astype(np.float64)
    mid = r.astype(ml_dtypes.bfloat16)
    lo = (r - mid.astype(np.float64)).astype(ml_dtypes.bfloat16)
    return hi, mid, lo


def _kd_perm(X, leaf):
    """Permutation sorting rows of X into contiguous leaves of size `leaf`
    via recursive median bisection (balanced: len(X) must be leaf * 2^k)."""
    out = []

    def rec(idx):
        if len(idx) <= leaf:
            out.append(idx)
            return
        P = X[idx]
        ax = int(np.argmax(P.max(0) - P.min(0)))
        order = np.argsort(P[:, ax], kind="stable")
        h = len(idx) // 2
        rec(idx[order[:h]])
        rec(idx[order[h:]])

    rec(np.arange(len(X)))
    return np.concatenate(out)


def _schedule(Xs, Xr, ell2):
    """Point-packed block-sparse schedule with per-slot tile budgets.
    Returns (perm_t, Ts, entries): Ts[s] is slot s's tile count (same on
    every core; descending); entries is a list of len(Ts)*8 (leaf_idx,
    train_point_array) pairs in (slot-major, core-minor) order, each array
    holding <= Ts[slot]*TT train indices (points within the cutoff of that
    test chunk's bbox); leaf_idx may repeat (split chunks) or be -1 (empty).
    """
    perm_t = _kd_perm(Xs, TC)
    Xs_s = Xs[perm_t]
    nt = M // TC
    tb_lo = Xs_s.reshape(nt, TC, 2).min(1)
    tb_hi = Xs_s.reshape(nt, TC, 2).max(1)
    cut2 = 2.0 * ell2 * TAU
    dx = np.maximum(0.0, np.maximum(tb_lo[:, None, 0] - Xr[None, :, 0],
                                    Xr[None, :, 0] - tb_hi[:, None, 0]))
    dy = np.maximum(0.0, np.maximum(tb_lo[:, None, 1] - Xr[None, :, 1],
                                    Xr[None, :, 1] - tb_hi[:, None, 1]))
    d2 = dx * dx + dy * dy
    need = d2 < cut2  # (nt, N)
    # Gathered points sorted near-to-far (by bbox distance) so the far
    # tail of every entry packs into whole tiles that the DVE fast-exp
    # can take over from ScalarE.  A chunk only qualifies for offload if
    # its worst-case exponent stays above the int16 underflow of the
    # fast-exp bit trick (diagonal-dependent).
    pts_of, near_of = [], []
    diag2 = ((tb_hi - tb_lo) ** 2).sum(1)
    zcut = 2.0 * ell2 * THETA
    for j in range(nt):
        p = np.nonzero(need[j])[0]
        dj = d2[j, p]
        order = np.argsort(dj, kind="stable")
        p = p[order]
        qual = (math.sqrt(cut2) + math.sqrt(diag2[j])) ** 2 < 160.0 * ell2
        near_of.append(int((dj[order] < zcut).sum()) if qual else len(p))
        pts_of.append(p)

    # split each chunk into balanced entries of <= cap tiles, sort descending,
    # deal 8 per slot: slot budget = max entry size in its rank-8 group.
    # Sweep the cap to minimize ACT cycles (T*TC cols + ~290 per ACT group).
    def build(cap):
        entries = []
        for j in range(nt):
            p = pts_of[j]
            n = max(1, math.ceil(len(p) / TT))
            k = math.ceil(n / cap)
            q, r = divmod(n, k)
            a = 0
            for i in range(k):
                sz = (q + 1 if i < r else q) * TT
                nn = max(0, min(near_of[j] - a, sz))
                entries.append((j, p[a:a + sz], nn))
                a += sz
        # sort by near-tile count first so each rank-8 group is homogeneous
        # in ACT tiles (nacts is a max over the group), then by size
        entries.sort(key=lambda e: (-math.ceil(e[2] / TT), -len(e[1])))
        while len(entries) % NCORES:
            entries.append((-1, np.array([], dtype=np.int64), 0))
        Ts, nacts = [], []
        for s in range(len(entries) // NCORES):
            grp = entries[s * NCORES:(s + 1) * NCORES]
            Ts.append(max(1, max(math.ceil(len(e[1]) / TT) for e in grp)))
            nacts.append(max(math.ceil(e[2] / TT) for e in grp))
        return Ts, nacts, entries

    best = None
    for cap in range(2 * G, 2, -1):
        Ts, nacts, entries = build(cap)
        cost = sum(Ts) * TC + 290 * sum(math.ceil(t / G) for t in Ts)
        if best is None or cost < best[0]:
            best = (cost, Ts, nacts, entries)
    _, Ts, nacts, entries = best

    # Weave the slot order so ACT-heavy and DVE-heavy slots alternate --
    # otherwise ScalarE finishes its slots first and the DVE tail runs with
    # ScalarE idle.  Only when the offload is active: otherwise keep the
    # size-descending order (big slots first overlaps best with the DMA).
    S = len(Ts)
    if sum(nacts) < sum(Ts):
        by_act = sorted(range(S), key=lambda s: -nacts[s])
        order = []
        lo, hi = 0, S - 1
        while lo <= hi:
            order.append(by_act[lo])
            if lo != hi:
                order.append(by_act[hi])
            lo += 1
            hi -= 1
        Ts = [Ts[s] for s in order]
        nacts = [nacts[s] for s in order]
        entries = [entries[s * NCORES + c] for s in order
                   for c in range(NCORES)]
    return perm_t, Ts, nacts, entries


def _build_program(bias, Ts, nacts):
    import concourse.mybir as mybir
    import concourse.tile as tile
    from concourse import bacc

    fp32 = mybir.dt.float32
    bf16 = mybir.dt.bfloat16
    i16 = mybir.dt.int16
    b_sch = 128.0 * (127.0 - C_SCH) + float(bias) * A_SCH
    S = len(Ts)
    Ws = [TC + t * TT for t in Ts]               # per-slot input columns
    offs = [0]
    for w in Ws:
        offs.append(offs[-1] + w)
    aoffs = [0]
    for t in Ts:
        aoffs.append(aoffs[-1] + t * 4)

    nc = bacc.Bacc(None, target_bir_lowering=False)
    IN_d = nc.declare_dram_parameter("inp", [KP, offs[-1]], bf16, isOutput=False)
    AL_d = nc.declare_dram_parameter("alp", [TT, aoffs[-1]], bf16, isOutput=False)
    OUT_d = nc.declare_dram_parameter("out", [4, S * TC], fp32, isOutput=True)

    # slot ranges for the NDMA grouped input transfers: group 0 is just
    # slot 0 (so the first exponent matmuls start as early as possible),
    # the rest balanced by columns
    bounds = [0, 1]
    for g in range(2, NDMA):
        tgt = offs[1] + (offs[-1] - offs[1]) * (g - 1) // (NDMA - 1)
        s = min(range(S + 1), key=lambda i: abs(offs[i] - tgt))
        bounds.append(max(bounds[-1] + 1, min(s, S - (NDMA - g))))
    bounds.append(S)

    with tile.TileContext(nc) as tc:
        with (
            tc.tile_pool(name="singles", bufs=1) as singles,
            tc.tile_pool(name="kpool", bufs=8) as kpool,
            tc.tile_pool(name="pse", bufs=2, space="PSUM") as pse,
            tc.tile_pool(name="psacc", bufs=2, space="PSUM") as psacc,
        ):
            # PE clock warmup: ~3.4us of junk matmuls on zeroed SBUF while
            # the input DMAs stream, so the HAM un-throttles before slot 0.
            warm_sb = singles.tile([128, TC], bf16, name="warm")
            nc.vector.memset(warm_sb, 0)
            warm_ps = psacc.tile([4, TC], fp32, name="acc")
            for w in range(NWARM):
                nc.tensor.matmul(
                    warm_ps, lhsT=warm_sb[:, :4], rhs=warm_sb,
                    start=(w == 0), stop=(w == NWARM - 1),
                )

            # All input triggers go on the sync queue in slot order (the 16
            # HW rings drain transfers in enqueue order, so slot 0's group
            # lands first); alpha rides the gpsimd queue so it isn't stuck
            # behind the bulk input in the ring FIFO.
            grp_of = {}
            gtiles = []
            al_sb = None
            for g in range(NDMA):
                s0, s1 = bounds[g], bounds[g + 1]
                t_in = singles.tile([KP, offs[s1] - offs[s0]], bf16,
                                    name=f"in{g}")
                nc.sync.dma_start(out=t_in, in_=IN_d[:, offs[s0]:offs[s1]])
                gtiles.append(t_in)
                for s in range(s0, s1):
                    grp_of[s] = (g, offs[s] - offs[s0])
                if g == 0:
                    al_sb = singles.tile([TT, aoffs[-1]], bf16, name="alp")
                    nc.gpsimd.dma_start(out=al_sb, in_=AL_d[:, :])

            osplit = (S - 2) * TC
            out_sb = singles.tile([4, osplit], fp32, name="osb")
            out_sb2 = singles.tile([4, S * TC - osplit], fp32, name="osb2")

            def emit_exp(s):
                # exponent matmuls for all groups of slot s; the near tiles
                # get a ScalarE exp, the far tail a DVE fast-exp (Schraudolph
                # int16 bit trick writing bf16 bits directly)
                T = Ts[s]
                g, base = grp_of[s]
                sb = gtiles[g]
                rhsB = sb[:, base:base + TC]
                ks = []
                for g0 in range(0, T, G):
                    g1 = min(g0 + G, T)
                    e = pse.tile([128, G * TC], fp32)
                    for t in range(g0, g1):
                        nc.tensor.matmul(
                            e[:, (t - g0) * TC:(t - g0 + 1) * TC],
                            lhsT=sb[:, base + TC + t * TT:base + TC + (t + 1) * TT],
                            rhs=rhsB,
                            start=True,
                            stop=True,
                        )
                    na = min(max(nacts[s] - g0, 0), g1 - g0)
                    ka = kf = None
                    if na > 0:
                        ka = kpool.tile([128, G * TC], bf16, name="ka")
                        nc.scalar.activation(
                            ka[:, :na * TC], e[:, :na * TC],
                            mybir.ActivationFunctionType.Exp, bias=float(bias)
                        )
                    if na < g1 - g0:
                        kf = kpool.tile([128, G * TC], bf16, name="kf")
                        nc.vector.tensor_scalar(
                            out=kf[:, na * TC:(g1 - g0) * TC].bitcast(i16),
                            in0=e[:, na * TC:(g1 - g0) * TC],
                            scalar1=A_SCH,
                            scalar2=b_sch,
                            op0=mybir.AluOpType.mult,
                            op1=mybir.AluOpType.add,
                        )
                    ks.append((ka, kf, na))
                return ks

            def emit_alpha(s, ks):
                # alpha contraction + output copy for slot s
                T = Ts[s]
                acc = psacc.tile([4, TC], fp32, name="acc")
                for gi, g0 in enumerate(range(0, T, G)):
                    g1 = min(g0 + G, T)
                    ka, kf, na = ks[gi]
                    for t in range(g0, g1):
                        k = ka if t - g0 < na else kf
                        nc.tensor.matmul(
                            acc,
                            lhsT=al_sb[:, aoffs[s] + t * 4:aoffs[s] + (t + 1) * 4],
                            rhs=k[:, (t - g0) * TC:(t - g0 + 1) * TC],
                            start=(t == 0),
                            stop=(t == T - 1),
                        )
                if s * TC < osplit:
                    nc.vector.tensor_copy(out_sb[:, s * TC:(s + 1) * TC], acc)
                else:
                    nc.vector.tensor_copy(
                        out_sb2[:, s * TC - osplit:(s + 1) * TC - osplit], acc
                    )

            # 1-deep software pipeline: exponent work runs one slot ahead of
            # the alpha contraction.  Going 2-deep measured +5.5us: with only
            # two PSUM exp buffers (bank-limited), the early-emitted exp
            # matmuls head-of-line block the PE queue waiting for the buffer
            # ACT still holds, while ready alpha work sits behind them.
            prev = emit_exp(0)
            for s in range(S):
                nxt = emit_exp(s + 1) if s + 1 < S else None
                emit_alpha(s, prev)
                prev = nxt
                if s == S - 3:
                    # first output chunk leaves while the last slots compute
                    nc.gpsimd.dma_start(out=OUT_d[:, :osplit], in_=out_sb)

            nc.gpsimd.dma_start(out=OUT_d[:, osplit:], in_=out_sb2)
    nc.compile()
    return nc


def _prep(X_test, X_train, alpha, log_lengthscale, log_outputscale):
    ell = np.exp(np.float32(log_lengthscale))
    ell2 = np.float64(np.float32(ell) ** 2)
    sf = np.exp(np.float32(log_outputscale))
    sf2 = np.float64(np.float32(sf) ** 2)
    bias = np.float32(np.log(sf2))

    perm_t, Ts, nacts, entries = _schedule(
        np.asarray(X_test, np.float64), np.asarray(X_train, np.float64), ell2
    )
    S = len(Ts)

    xt = X_train.astype(np.float64)
    xs = X_test.astype(np.float64)[perm_t]
    al = alpha.astype(np.float64)

    # Train-side feature matrix A (C, N), original train order
    x0h, x0l = _split2(xt[:, 0])
    x1h, x1l = _split2(xt[:, 1])
    pj = -(xt[:, 0] ** 2 + xt[:, 1] ** 2) / (2.0 * ell2)
    pjh, pjm, pjl = _split3(pj)
    ones = np.ones(N, dtype=ml_dtypes.bfloat16)
    A = np.stack(
        [ones, ones, ones, x0h, x0h, x0l, x0l, x1h, x1h, x1l, x1l, pjh, pjm, pjl]
    )
    # sentinel all-zero feature column for padding slots: exponent 0 -> K=1,
    # contributes nothing (alpha 0) and never underflows the DVE fast-exp
    A = np.concatenate([A, np.zeros((C, 1), dtype=ml_dtypes.bfloat16)], axis=1)

    # Test-side feature matrix B (C, M), kd-sorted test order
    T0 = -(xs[:, 0] ** 2 + xs[:, 1] ** 2) / (2.0 * ell2)
    T0h, T0m, T0l = _split3(T0)
    u0 = xs[:, 0] / ell2
    u0h, u0l = _split2(u0)
    u1 = xs[:, 1] / ell2
    u1h, u1l = _split2(u1)
    onesM = np.ones(M, dtype=ml_dtypes.bfloat16)
    B = np.stack(
        [T0h, T0m, T0l, u0h, u0l, u0h, u0l, u1h, u1l, u1h, u1l, onesM, onesM, onesM]
    )

    # alpha (N, 4): hi/lo split of each alpha column, original train order
    arh, arl = _split2(al[:, 0])
    aih, ail = _split2(al[:, 1])
    AL = np.stack([arh, arl, aih, ail], axis=1)
    AL = np.concatenate([AL, np.zeros((1, 4), dtype=ml_dtypes.bfloat16)], axis=0)

    # Gather per-core inputs from the schedule: per-slot [B | A] and alpha
    Ws = [TC + t * TT for t in Ts]
    offs = [0]
    for w in Ws:
        offs.append(offs[-1] + w)
    aoffs = [0]
    for t in Ts:
        aoffs.append(aoffs[-1] + t * 4)
    in_maps, placements = [], []
    for c in range(NCORES):
        IN_g = np.zeros((KP, offs[-1]), dtype=ml_dtypes.bfloat16)
        AL_g = np.zeros((TT, aoffs[-1]), dtype=ml_dtypes.bfloat16)
        place = []
        for s in range(S):
            T = Ts[s]
            leaf, pts, _nn = entries[s * NCORES + c]
            bleaf = leaf if leaf >= 0 else 0
            col = offs[s]
            IN_g[:C, col:col + TC] = B[:, bleaf * TC:(bleaf + 1) * TC]
            place.append(leaf)
            n = len(pts)
            pad = np.full(T * TT - n, N, dtype=np.int64)
            full = np.concatenate([pts, pad]) if n < T * TT else pts
            IN_g[:C, col + TC:col + TC + T * TT] = A[:, full]
            alg = AL[full]              # (T*TT, 4)
            alg[n:] = 0
            AL_g[:, aoffs[s]:aoffs[s + 1]] = (
                alg.reshape(T, TT, 4).transpose(1, 0, 2).reshape(TT, T * 4)
            )
        in_maps.append({"inp": IN_g, "alp": AL_g})
        placements.append(place)
    return in_maps, placements, perm_t, Ts, nacts, bias


def _combine(results, placements, perm_t, S):
    mu_sorted = np.zeros((M, 2), dtype=np.float32)
    for c in range(NCORES):
        o = results[c]["out"]
        for s, leaf in enumerate(placements[c]):
            if leaf < 0:
                continue
            sl = slice(leaf * TC, (leaf + 1) * TC)
            mu_sorted[sl, 0] += o[0, s * TC:(s + 1) * TC] + o[1, s * TC:(s + 1) * TC]
            mu_sorted[sl, 1] += o[2, s * TC:(s + 1) * TC] + o[3, s * TC:(s + 1) * TC]
    out = np.empty((M, 2), dtype=np.float32)
    out[perm_t] = mu_sorted
    return out


def kernel(X_test, X_train, alpha, log_lengthscale, log_outputscale):
    from concourse.bass_utils import run_bass_kernel_spmd

    in_maps, placements, perm_t, Ts, nacts, bias = _prep(
        np.asarray(X_test), np.asarray(X_train), np.asarray(alpha),
        np.asarray(log_lengthscale), np.asarray(log_outputscale)
    )
    key = (tuple(Ts), tuple(nacts), float(bias))
    if key not in _cache:
        _cache[key] = _build_program(bias, Ts, nacts)
    nc = _cache[key]

    core_ids = list(range(NCORES))
    res = run_bass_kernel_spmd(nc, in_maps, core_ids)
    return _combine(res.results, placements, perm_t, len(Ts))


# revision 55
# speedup vs baseline: 1.0680x; 1.0680x over previous
"""GP posterior mean mu = K_rbf(X_test, X_train) @ alpha on 8 NeuronCores,
exploiting the locality of the RBF kernel (lengthscale 0.1 on N(0,1) data).

Math per block: K[j,i] = sf2 * exp(-0.5*||xt_i - x_j||^2 / ell2), with the
exponent expressed as a single 14-term dot product built from bf16 hi/lo
splits of the fp32 operands, zero-padded to a 128-deep contraction.  The
padding is NOT for streaming speed (a small-K matmul streams at the same
N-cycles rate) -- it keeps the PE HAM activity monitor fed: a 16-row
contraction reads as a near-idle array and the clock gate locks the PE at
1.2 GHz instead of 2.4 GHz (measured 2x on every matmul).  ScalarE applies
exp (sf2 folded into the activation bias), and a second TensorE matmul
contracts K against hi/lo-split alpha, accumulating in PSUM.

Sparsity: test points are sorted into 64 compact spatial chunks of 256 by
recursive median bisection (host side).  For each chunk, only the train
points whose distance to the chunk's bbox keeps the RBF exponent above -TAU
are gathered (sorted near-to-far) and packed densely into per-chunk 128-point
contraction tiles.  The SPMD program is a uniform grid of S slots x T
train-tiles per core; heavy chunks are split across slots (partial sums
merged on host) and the remainder is padded with a zero-feature sentinel
column (exp -> 1, alpha 0), so every core runs the identical instruction
stream and only the gathered tile DATA differs per core.

Schedule learned from HW traces:
- Few, ordered DMAs: per-dma_start queue cost is ~0.6us and the 16 HW rings
  drain transfers in enqueue order, so slot 0's group goes first (tiny) and
  alpha rides a different trigger queue to bypass the bulk input FIFO.
- ~3.6us of junk matmuls on a zeroed tile run during the input DMA to flip
  the HAM clock gate before real work starts.
- One output buffer, two DMAs (the bulk leaves while the last slot runs).
- A fixed ~10us runtime epilogue (semaphore drain storm + final barrier)
  exists regardless of program content; a do-nothing program measures
  ~13.8us through this same path.
"""

import math

import numpy as np
import ml_dtypes

M = 16384
N = 16384
NCORES = 8
TC = 256                  # test points per chunk (columns per tile)
TT = 128                  # train points per tile (one PE contraction)
TAU = 3.2                 # drop blocks with min exponent magnitude > TAU
G = 6                     # train tiles covered by one ACT instruction
C = 14                    # used contraction rows of the exponent matmul
KP = 128                  # shipped contraction rows (128: a small-K matmul
                          # starves the PE HAM activity monitor and locks the
                          # clock at 1.2 GHz, so pad the contraction to 128)
NDMA = 5                  # input DMA groups (first kept tiny for fast start)
NWARM = 18                # junk matmuls issued during the input DMA to flip
                          # the PE clock gate to 2.4 GHz before real work.
                          # Must cover the full ~3.4us HAM window by itself:
                          # shortening to 12 (hoping slot 0's matmuls finish
                          # the streak) left the whole stream at 1.2 GHz
                          # (measured +7us) -- the idle gap before slot 0's
                          # data resets the activity window
THETA = 3.5               # tiles whose points all sit beyond this exponent
                          # magnitude get a DVE fast-exp instead of ScalarE.
                          # Set to TAU = disabled: measured on HW, the DVE
                          # offload never beat ScalarE because TensorE is
                          # equally saturated (~22us) and the coarse DVE
                          # instruction granularity added coupling stalls
C_SCH = 0.0434            # Schraudolph minimax offset for the fast exp
A_SCH = 128.0 * 1.4426950408889634      # 2^7 * log2(e)

_cache = {}


def _split2(v):
    hi = v.astype(ml_dtypes.bfloat16)
    lo = (v - hi.astype(np.float64)).astype(ml_dtypes.bfloat16)
    return hi, lo


def _split3(v):
    hi = v.astype(ml_dtypes.bfloat16)
    r = v - hi.astype(np.float64)
    mid = r.astype(ml_dtypes.bfloat16)
    lo = (r - mid.astype(np.float64)).astype(ml_dtypes.bfloat16)
    return hi, mid, lo


def _kd_perm(X, leaf):
    """Permutation sorting rows of X into contiguous leaves of size `leaf`
    via recursive median bisection (balanced: len(X) must be leaf * 2^k)."""
    out = []

    def rec(idx):
        if len(idx) <= leaf:
            out.append(idx)
            return
        P = X[idx]
        ax = int(np.argmax(P.max(0) - P.min(0)))
        order = np.argsort(P[:, ax], kind="stable")
        h = len(idx) // 2
        rec(idx[order[:h]])
        rec(idx[order[h:]])

    rec(np.arange(len(X)))
    return np.concatenate(out)


def _schedule(Xs, Xr, ell2):
    """Point-packed block-sparse schedule with per-slot tile budgets.
    Returns (perm_t, Ts, entries): Ts[s] is slot s's tile count (same on
    every core; descending); entries is a list of len(Ts)*8 (leaf_idx,
    train_point_array) pairs in (slot-major, core-minor) order, each array
    holding <= Ts[slot]*TT train indices (points within the cutoff of that
    test chunk's bbox); leaf_idx may repeat (split chunks) or be -1 (empty).
    """
    perm_t = _kd_perm(Xs, TC)
    Xs_s = Xs[perm_t]
    nt = M // TC
    tb_lo = Xs_s.reshape(nt, TC, 2).min(1)
    tb_hi = Xs_s.reshape(nt, TC, 2).max(1)
    cut2 = 2.0 * ell2 * TAU
    dx = np.maximum(0.0, np.maximum(tb_lo[:, None, 0] - Xr[None, :, 0],
                                    Xr[None, :, 0] - tb_hi[:, None, 0]))
    dy = np.maximum(0.0, np.maximum(tb_lo[:, None, 1] - Xr[None, :, 1],
                                    Xr[None, :, 1] - tb_hi[:, None, 1]))
    d2 = dx * dx + dy * dy
    need = d2 < cut2  # (nt, N)
    # Gathered points sorted near-to-far (by bbox distance) so the far
    # tail of every entry packs into whole tiles that the DVE fast-exp
    # can take over from ScalarE.  A chunk only qualifies for offload if
    # its worst-case exponent stays above the int16 underflow of the
    # fast-exp bit trick (diagonal-dependent).
    pts_of, near_of = [], []
    diag2 = ((tb_hi - tb_lo) ** 2).sum(1)
    zcut = 2.0 * ell2 * THETA
    for j in range(nt):
        p = np.nonzero(need[j])[0]
        dj = d2[j, p]
        order = np.argsort(dj, kind="stable")
        p = p[order]
        qual = (math.sqrt(cut2) + math.sqrt(diag2[j])) ** 2 < 160.0 * ell2
        near_of.append(int((dj[order] < zcut).sum()) if qual else len(p))
        pts_of.append(p)

    # split each chunk into balanced entries of <= cap tiles, sort descending,
    # deal 8 per slot: slot budget = max entry size in its rank-8 group.
    # Sweep the cap to minimize ACT cycles (T*TC cols + ~290 per ACT group).
    def build(cap):
        entries = []
        for j in range(nt):
            p = pts_of[j]
            n = max(1, math.ceil(len(p) / TT))
            k = math.ceil(n / cap)
            q, r = divmod(n, k)
            a = 0
            for i in range(k):
                sz = (q + 1 if i < r else q) * TT
                nn = max(0, min(near_of[j] - a, sz))
                entries.append((j, p[a:a + sz], nn))
                a += sz
        # sort by near-tile count first so each rank-8 group is homogeneous
        # in ACT tiles (nacts is a max over the group), then by size
        entries.sort(key=lambda e: (-math.ceil(e[2] / TT), -len(e[1])))
        while len(entries) % NCORES:
            entries.append((-1, np.array([], dtype=np.int64), 0))
        Ts, nacts = [], []
        for s in range(len(entries) // NCORES):
            grp = entries[s * NCORES:(s + 1) * NCORES]
            Ts.append(max(1, max(math.ceil(len(e[1]) / TT) for e in grp)))
            nacts.append(max(math.ceil(e[2] / TT) for e in grp))
        return Ts, nacts, entries

    best = None
    for cap in range(2 * G, 2, -1):
        Ts, nacts, entries = build(cap)
        cost = sum(Ts) * TC + 290 * sum(math.ceil(t / G) for t in Ts)
        if best is None or cost < best[0]:
            best = (cost, Ts, nacts, entries)
    _, Ts, nacts, entries = best

    # Weave the slot order so ACT-heavy and DVE-heavy slots alternate --
    # otherwise ScalarE finishes its slots first and the DVE tail runs with
    # ScalarE idle.  Only when the offload is active: otherwise keep the
    # size-descending order (big slots first overlaps best with the DMA).
    S = len(Ts)
    if sum(nacts) < sum(Ts):
        by_act = sorted(range(S), key=lambda s: -nacts[s])
        order = []
        lo, hi = 0, S - 1
        while lo <= hi:
            order.append(by_act[lo])
            if lo != hi:
                order.append(by_act[hi])
            lo += 1
            hi -= 1
        Ts = [Ts[s] for s in order]
        nacts = [nacts[s] for s in order]
        entries = [entries[s * NCORES + c] for s in order
                   for c in range(NCORES)]
    return perm_t, Ts, nacts, entries


def _build_program(bias, Ts, nacts):
    import concourse.mybir as mybir
    import concourse.tile as tile
    from concourse import bacc

    fp32 = mybir.dt.float32
    bf16 = mybir.dt.bfloat16
    i16 = mybir.dt.int16
    b_sch = 128.0 * (127.0 - C_SCH) + float(bias) * A_SCH
    S = len(Ts)
    Ws = [TC + t * TT for t in Ts]               # per-slot input columns
    offs = [0]
    for w in Ws:
        offs.append(offs[-1] + w)
    aoffs = [0]
    for t in Ts:
        aoffs.append(aoffs[-1] + t * 4)

    nc = bacc.Bacc(None, target_bir_lowering=False)
    IN_d = nc.declare_dram_parameter("inp", [KP, offs[-1]], bf16, isOutput=False)
    AL_d = nc.declare_dram_parameter("alp", [TT, aoffs[-1]], bf16, isOutput=False)
    OUT_d = nc.declare_dram_parameter("out", [4, S * TC], fp32, isOutput=True)

    # slot ranges for the NDMA grouped input transfers: group 0 is just
    # slot 0 (so the first exponent matmuls start as early as possible),
    # the rest balanced by columns
    bounds = [0, 1]
    for g in range(2, NDMA):
        tgt = offs[1] + (offs[-1] - offs[1]) * (g - 1) // (NDMA - 1)
        s = min(range(S + 1), key=lambda i: abs(offs[i] - tgt))
        bounds.append(max(bounds[-1] + 1, min(s, S - (NDMA - g))))
    bounds.append(S)

    with tile.TileContext(nc) as tc:
        with (
            tc.tile_pool(name="singles", bufs=1) as singles,
            tc.tile_pool(name="kpool", bufs=8) as kpool,
            tc.tile_pool(name="pse", bufs=2, space="PSUM") as pse,
            tc.tile_pool(name="psacc", bufs=2, space="PSUM") as psacc,
        ):
            # PE clock warmup: ~3.4us of junk matmuls on zeroed SBUF while
            # the input DMAs stream, so the HAM un-throttles before slot 0.
            warm_sb = singles.tile([128, TC], bf16, name="warm")
            nc.vector.memset(warm_sb, 0)
            warm_ps = psacc.tile([4, TC], fp32, name="acc")
            for w in range(NWARM):
                nc.tensor.matmul(
                    warm_ps, lhsT=warm_sb[:, :4], rhs=warm_sb,
                    start=(w == 0), stop=(w == NWARM - 1),
                )

            # All input triggers go on the sync queue in slot order (the 16
            # HW rings drain transfers in enqueue order, so slot 0's group
            # lands first); alpha rides the gpsimd queue so it isn't stuck
            # behind the bulk input in the ring FIFO.
            grp_of = {}
            gtiles = []
            al_sb = None
            for g in range(NDMA):
                s0, s1 = bounds[g], bounds[g + 1]
                t_in = singles.tile([KP, offs[s1] - offs[s0]], bf16,
                                    name=f"in{g}")
                nc.sync.dma_start(out=t_in, in_=IN_d[:, offs[s0]:offs[s1]])
                gtiles.append(t_in)
                for s in range(s0, s1):
                    grp_of[s] = (g, offs[s] - offs[s0])
                if g == 0:
                    al_sb = singles.tile([TT, aoffs[-1]], bf16, name="alp")
                    nc.gpsimd.dma_start(out=al_sb, in_=AL_d[:, :])

            osplit = (S - 2) * TC
            out_sb = singles.tile([4, osplit], fp32, name="osb")
            out_sb2 = singles.tile([4, S * TC - osplit], fp32, name="osb2")

            def gstep(s):
                # slot 0's ACT is split in half so the very first EXP only
                # waits for 3 exponent matmuls after the warmup gate (the
                # extra ACT overhead lands while ScalarE is otherwise idle)
                return 3 if s == 0 else G

            def emit_exp(s):
                # exponent matmuls for all groups of slot s; the near tiles
                # get a ScalarE exp, the far tail a DVE fast-exp (Schraudolph
                # int16 bit trick writing bf16 bits directly)
                T = Ts[s]
                g, base = grp_of[s]
                sb = gtiles[g]
                rhsB = sb[:, base:base + TC]
                ks = []
                st = gstep(s)
                for g0 in range(0, T, st):
                    g1 = min(g0 + st, T)
                    e = pse.tile([128, G * TC], fp32)
                    for t in range(g0, g1):
                        nc.tensor.matmul(
                            e[:, (t - g0) * TC:(t - g0 + 1) * TC],
                            lhsT=sb[:, base + TC + t * TT:base + TC + (t + 1) * TT],
                            rhs=rhsB,
                            start=True,
                            stop=True,
                        )
                    na = min(max(nacts[s] - g0, 0), g1 - g0)
                    ka = kf = None
                    if na > 0:
                        ka = kpool.tile([128, G * TC], bf16, name="ka")
                        nc.scalar.activation(
                            ka[:, :na * TC], e[:, :na * TC],
                            mybir.ActivationFunctionType.Exp, bias=float(bias)
                        )
                    if na < g1 - g0:
                        kf = kpool.tile([128, G * TC], bf16, name="kf")
                        nc.vector.tensor_scalar(
                            out=kf[:, na * TC:(g1 - g0) * TC].bitcast(i16),
                            in0=e[:, na * TC:(g1 - g0) * TC],
                            scalar1=A_SCH,
                            scalar2=b_sch,
                            op0=mybir.AluOpType.mult,
                            op1=mybir.AluOpType.add,
                        )
                    ks.append((ka, kf, na))
                return ks

            def emit_alpha(s, ks):
                # alpha contraction + output copy for slot s
                T = Ts[s]
                acc = psacc.tile([4, TC], fp32, name="acc")
                st = gstep(s)
                for gi, g0 in enumerate(range(0, T, st)):
                    g1 = min(g0 + st, T)
                    ka, kf, na = ks[gi]
                    for t in range(g0, g1):
                        k = ka if t - g0 < na else kf
                        nc.tensor.matmul(
                            acc,
                            lhsT=al_sb[:, aoffs[s] + t * 4:aoffs[s] + (t + 1) * 4],
                            rhs=k[:, (t - g0) * TC:(t - g0 + 1) * TC],
                            start=(t == 0),
                            stop=(t == T - 1),
                        )
                if s * TC < osplit:
                    nc.vector.tensor_copy(out_sb[:, s * TC:(s + 1) * TC], acc)
                else:
                    nc.vector.tensor_copy(
                        out_sb2[:, s * TC - osplit:(s + 1) * TC - osplit], acc
                    )

            # 1-deep software pipeline: exponent work runs one slot ahead of
            # the alpha contraction.  Going 2-deep measured +5.5us: with only
            # two PSUM exp buffers (bank-limited), the early-emitted exp
            # matmuls head-of-line block the PE queue waiting for the buffer
            # ACT still holds, while ready alpha work sits behind them.
            prev = emit_exp(0)
            for s in range(S):
                nxt = emit_exp(s + 1) if s + 1 < S else None
                emit_alpha(s, prev)
                prev = nxt
                if s == S - 3:
                    # first output chunk leaves while the last slots compute
                    nc.gpsimd.dma_start(out=OUT_d[:, :osplit], in_=out_sb)

            nc.gpsimd.dma_start(out=OUT_d[:, osplit:], in_=out_sb2)
    nc.compile()
    return nc


def _prep(X_test, X_train, alpha, log_lengthscale, log_outputscale):
    ell = np.exp(np.float32(log_lengthscale))
    ell2 = np.float64(np.float32(ell) ** 2)
    sf = np.exp(np.float32(log_outputscale))
    sf2 = np.float64(np.float32(sf) ** 2)
    bias = np.float32(np.log(sf2))

    perm_t, Ts, nacts, entries = _schedule(
        np.asarray(X_test, np.float64), np.asarray(X_train, np.float64), ell2
    )
    S = len(Ts)

    xt = X_train.astype(np.float64)
    xs = X_test.astype(np.float64)[perm_t]
    al = alpha.astype(np.float64)

    # Train-side feature matrix A (C, N), original train order
    x0h, x0l = _split2(xt[:, 0])
    x1h, x1l = _split2(xt[:, 1])
    pj = -(xt[:, 0] ** 2 + xt[:, 1] ** 2) / (2.0 * ell2)
    pjh, pjm, pjl = _split3(pj)
    ones = np.ones(N, dtype=ml_dtypes.bfloat16)
    A = np.stack(
        [ones, ones, ones, x0h, x0h, x0l, x0l, x1h, x1h, x1l, x1l, pjh, pjm, pjl]
    )
    # sentinel all-zero feature column for padding slots: exponent 0 -> K=1,
    # contributes nothing (alpha 0) and never underflows the DVE fast-exp
    A = np.concatenate([A, np.zeros((C, 1), dtype=ml_dtypes.bfloat16)], axis=1)

    # Test-side feature matrix B (C, M), kd-sorted test order
    T0 = -(xs[:, 0] ** 2 + xs[:, 1] ** 2) / (2.0 * ell2)
    T0h, T0m, T0l = _split3(T0)
    u0 = xs[:, 0] / ell2
    u0h, u0l = _split2(u0)
    u1 = xs[:, 1] / ell2
    u1h, u1l = _split2(u1)
    onesM = np.ones(M, dtype=ml_dtypes.bfloat16)
    B = np.stack(
        [T0h, T0m, T0l, u0h, u0l, u0h, u0l, u1h, u1l, u1h, u1l, onesM, onesM, onesM]
    )

    # alpha (N, 4): hi/lo split of each alpha column, original train order
    arh, arl = _split2(al[:, 0])
    aih, ail = _split2(al[:, 1])
    AL = np.stack([arh, arl, aih, ail], axis=1)
    AL = np.concatenate([AL, np.zeros((1, 4), dtype=ml_dtypes.bfloat16)], axis=0)

    # Gather per-core inputs from the schedule: per-slot [B | A] and alpha
    Ws = [TC + t * TT for t in Ts]
    offs = [0]
    for w in Ws:
        offs.append(offs[-1] + w)
    aoffs = [0]
    for t in Ts:
        aoffs.append(aoffs[-1] + t * 4)
    in_maps, placements = [], []
    for c in range(NCORES):
        IN_g = np.zeros((KP, offs[-1]), dtype=ml_dtypes.bfloat16)
        AL_g = np.zeros((TT, aoffs[-1]), dtype=ml_dtypes.bfloat16)
        place = []
        for s in range(S):
            T = Ts[s]
            leaf, pts, _nn = entries[s * NCORES + c]
            bleaf = leaf if leaf >= 0 else 0
            col = offs[s]
            IN_g[:C, col:col + TC] = B[:, bleaf * TC:(bleaf + 1) * TC]
            place.append(leaf)
            n = len(pts)
            pad = np.full(T * TT - n, N, dtype=np.int64)
            full = np.concatenate([pts, pad]) if n < T * TT else pts
            IN_g[:C, col + TC:col + TC + T * TT] = A[:, full]
            alg = AL[full]              # (T*TT, 4)
            alg[n:] = 0
            AL_g[:, aoffs[s]:aoffs[s + 1]] = (
                alg.reshape(T, TT, 4).transpose(1, 0, 2).reshape(TT, T * 4)
            )
        in_maps.append({"inp": IN_g, "alp": AL_g})
        placements.append(place)
    return in_maps, placements, perm_t, Ts, nacts, bias


def _combine(results, placements, perm_t, S):
    mu_sorted = np.zeros((M, 2), dtype=np.float32)
    for c in range(NCORES):
        o = results[c]["out"]
        for s, leaf in enumerate(placements[c]):
            if leaf < 0:
                continue
            sl = slice(leaf * TC, (leaf + 1) * TC)
            mu_sorted[sl, 0] += o[0, s * TC:(s + 1) * TC] + o[1, s * TC:(s + 1) * TC]
            mu_sorted[sl, 1] += o[2, s * TC:(s + 1) * TC] + o[3, s * TC:(s + 1) * TC]
    out = np.empty((M, 2), dtype=np.float32)
    out[perm_t] = mu_sorted
    return out


def kernel(X_test, X_train, alpha, log_lengthscale, log_outputscale):
    from concourse.bass_utils import run_bass_kernel_spmd

    in_maps, placements, perm_t, Ts, nacts, bias = _prep(
        np.asarray(X_test), np.asarray(X_train), np.asarray(alpha),
        np.asarray(log_lengthscale), np.asarray(log_outputscale)
    )
    key = (tuple(Ts), tuple(nacts), float(bias))
    if key not in _cache:
        _cache[key] = _build_program(bias, Ts, nacts)
    nc = _cache[key]

    core_ids = list(range(NCORES))
    res = run_bass_kernel_spmd(nc, in_maps, core_ids)
    return _combine(res.results, placements, perm_t, len(Ts))
